# revision 18
# baseline (speedup 1.0000x reference)
"""Trainium2 Bass kernel for nn_FCVI_Net_78864189489850.

Computation (reference):
  L = lower-tri scatter of cov_vector (exp on diag)           [769, 769]
  samples = mean + L @ z                                      [769, S, B]
  W0 = samples[0:256], b0 = samples[256:512],
  W1 = samples[512:768], b1 = samples[768]
  out = sum_o relu(x*W0 + b0) * W1 + b1                       [S, B]

Strategy (8 NeuronCores, batch-sharded, no cross-device comms):
  - Everything is fused into ONE PSUM bank per 128-column tile:
      pub[:, 0:256]   = u2 = x*s0 + s1          (via host-prescaled xz rows)
      pub[:, 256:512] = v  = s2 + mean2         (mean2 via a K=1 ones matmul)
  - 9 matmuls per c-tile with block-triangular trimming (2944 cycles):
      z-t0..t2 -> N=512, z-t3 -> N=384, xz-t0 -> N=256, xz-t1 -> N=128,
      ones x mean2 -> N=256, z-t4 -> N=256, z-t5 -> N=128.
  - DVE: u2a = pub0 + apar (apar = x*mean0+mean1, host-built, 2 parities),
         g = max(u2a, 0) * pub1 with accum_out -> stag column  (relu fused!)
  - b1 row (s3 = L[768,:]@z + mean[768]) is computed fully on host and
    added after the device pass; scalar engine and gpsimd are unused.
  - z ships as f16 in DMA-friendly [chunk][part][slot][col] layout
    (8KB contiguous per partition per chunk); LT ships as one packed
    [128, 2688] f16 image of exactly the needed column ranges.
  - ~8 junk warm-up matmuls at t=0 (no DMA deps) spin the PE HAM clock
    up to 2.4 GHz while the first z chunk streams in.
"""
import os
import numpy as np

P = 769
S = 16
B = 2048
NCORES = 8
BC = B // NCORES          # 256 batch per core
NCOL = S * BC             # 4096 columns per core
NCT = NCOL // 128         # 32 c-tiles per core
NCHUNK = 8                # z DMA chunks
CHW = NCOL // NCHUNK      # 512

NWARM = int(os.environ.get("BASS_FCVI_WARM", "9"))

# lt image segments: (k0, i0, i1) in MM issue order; offsets accumulate
LT_SEGS = [
    (0,   256, 768),   # z-t0  -> pub[:, 0:512]
    (128, 256, 768),   # z-t1  -> pub[:, 0:512]
    (256, 256, 768),   # z-t2  -> pub[:, 0:512]
    (384, 384, 768),   # z-t3  -> pub[:, 128:512]
    (512, 512, 768),   # z-t4  -> pub[:, 256:512]
    (640, 640, 768),   # z-t5  -> pub[:, 384:512]
    (0,   0,   256),   # xz-t0 -> pub[:, 0:256]
    (128, 128, 256),   # xz-t1 -> pub[:, 128:256]
]
LT_OFF = []
_o = 0
for _k0, _i0, _i1 in LT_SEGS:
    LT_OFF.append(_o)
    _o += _i1 - _i0
LT_W = _o  # 2688

_cache = {}


def _build_program():
    import concourse.bacc as bacc
    import concourse.tile as tile
    from concourse import mybir

    f16 = mybir.dt.float16
    f32 = mybir.dt.float32

    nc = bacc.Bacc("TRN2", target_bir_lowering=False, debug=False)

    za0_d = nc.dram_tensor("za0", [2, 128, 8, 256], f16, kind="ExternalInput")
    za_d = nc.dram_tensor("za", [NCHUNK - 1, 128, 8, CHW], f16, kind="ExternalInput")
    lt_d = nc.dram_tensor("lt", [128, LT_W], f16, kind="ExternalInput")
    ap_d = nc.dram_tensor("apm", [128, 2, 256], f32, kind="ExternalInput")
    m2_d = nc.dram_tensor("m2", [128, 256], f16, kind="ExternalInput")
    out_d = nc.dram_tensor("out", [128, NCT], f32, kind="ExternalOutput")
    LT_HEAD = 1920  # lt cols for z-t0..t3 (tile 0's first four matmuls)

    with tile.TileContext(nc) as tc:
        with (
            tc.tile_pool(name="zpool", bufs=1) as zpool,
            tc.tile_pool(name="cpool", bufs=1) as cpool,
            tc.tile_pool(name="upool", bufs=3) as upool,
            tc.tile_pool(name="gpool", bufs=2) as gpool,
            tc.tile_pool(name="pub", bufs=7, space="PSUM") as pub_pool,
            tc.tile_pool(name="pwarm", bufs=1, space="PSUM") as pwarm_pool,
        ):
            # --- PE warm-up: junk matmuls with no DMA deps spin HAM to 2.4GHz
            warm = cpool.tile([128, 384], f16, tag="warm")
            nc.vector.memset(warm[:], 0.0)
            # e0: row 0 = ones, rest 0 -> lhsT for the mean2 broadcast matmul
            e0 = cpool.tile([128, 128], f16, tag="e0")
            nc.vector.memset(e0[:], 0.0)
            nc.vector.memset(e0[0:1, :], 1.0)
            pw = pwarm_pool.tile([128, 512], f32, tag="pw")
            for _ in range(NWARM):
                nc.tensor.matmul(pw[:, 0:256], warm[:, 0:128], warm[:, 128:384],
                                 start=True, stop=True)

            # --- DMAs: tile 0's deps first, issue spread over two queues ---
            zc = [None] * NCHUNK
            zc0 = [None, None]
            ltt = cpool.tile([128, LT_W], f16, tag="lt")
            apm = cpool.tile([128, 2, 256], f32, tag="apm")
            m2t = cpool.tile([128, 256], f16, tag="m2")

            for h in range(2):
                zht = zpool.tile([128, 8, 256], f16, tag=f"zc0{h}")
                zc0[h] = zht
            for q in range(1, NCHUNK):
                zqt = zpool.tile([128, 8, CHW], f16, tag=f"zc{q}")
                zc[q] = zqt

            # Two DGE rings (~220 GB/s each) pulled in parallel; both work
            # the tile-0 critical path first: sync ring streams z bytes,
            # scalar ring streams lt/mean bytes. Chunks then alternate.
            nc.sync.dma_start(out=zc0[0][:, 0:4, :], in_=za0_d.ap()[0, :, 0:4, :])
            nc.scalar.dma_start(out=ltt[:, 0:LT_HEAD], in_=lt_d.ap()[:, 0:LT_HEAD])
            nc.sync.dma_start(out=zc0[0][:, 4:8, :], in_=za0_d.ap()[0, :, 4:8, :])
            nc.scalar.dma_start(out=m2t[:], in_=m2_d.ap()[:, :])
            nc.scalar.dma_start(out=ltt[:, LT_HEAD:LT_W],
                                in_=lt_d.ap()[:, LT_HEAD:LT_W])
            nc.sync.dma_start(out=zc0[1][:], in_=za0_d.ap()[1])
            nc.scalar.dma_start(out=apm[:], in_=ap_d.ap()[:, :, :])
            for q in range(1, NCHUNK):
                eng = nc.scalar if q % 2 == 1 else nc.sync
                eng.dma_start(out=zc[q][:], in_=za_d.ap()[q - 1, :, :, :])

            stag = cpool.tile([128, NCT], f32, tag="stag")

            def lhs(m, s):
                q, cl = divmod(m * 128, CHW)
                if q == 0:
                    return zc0[m // 2][:, s, (m % 2) * 128:(m % 2) * 128 + 128]
                return zc[q][:, s, cl:cl + 128]

            def seg(t):
                return ltt[:, LT_OFF[t]:LT_OFF[t] + (LT_SEGS[t][2] - LT_SEGS[t][1])]

            MM = nc.tensor.matmul
            for m in range(NCT):
                pub = pub_pool.tile([128, 512], f32, tag="pub")
                MM(pub[:, 0:512],   lhs(m, 0), seg(0), start=True, stop=False)
                MM(pub[:, 0:512],   lhs(m, 1), seg(1), start=False, stop=False)
                MM(pub[:, 0:512],   lhs(m, 2), seg(2), start=False, stop=False)
                MM(pub[:, 128:512], lhs(m, 3), seg(3), start=False, stop=False)
                MM(pub[:, 0:256],   lhs(m, 6), seg(6), start=False, stop=False)
                MM(pub[:, 128:256], lhs(m, 7), seg(7), start=False, stop=False)
                MM(pub[:, 256:512], e0[:, :], m2t[:, :], start=False, stop=False)
                MM(pub[:, 256:512], lhs(m, 4), seg(4), start=False, stop=False)
                MM(pub[:, 384:512], lhs(m, 5), seg(5), start=False, stop=True)

                u2 = upool.tile([128, 256], f32, tag="u2")
                nc.vector.tensor_add(u2[:], pub[:, 0:256], apm[:, m % 2, :])
                g = gpool.tile([128, 256], f32, tag="g")
                nc.vector.scalar_tensor_tensor(
                    out=g[:], in0=u2[:], scalar=0.0, in1=pub[:, 256:512],
                    op0=mybir.AluOpType.max, op1=mybir.AluOpType.mult,
                    accum_out=stag[:, m:m + 1])

                if m in (NCT // 2 - 1, NCT - 1):
                    h_ = 0 if m == NCT // 2 - 1 else 1
                    sl = slice(h_ * (NCT // 2), (h_ + 1) * (NCT // 2))
                    nc.sync.dma_start(out=out_d.ap()[:, sl], in_=stag[:, sl])

    nc.compile()
    return nc


def _prep_inputs(x, mean, cov_vector, z):
    f16 = np.float16

    L = np.zeros((P, P), dtype=np.float32)
    L[np.tril_indices(P)] = cov_vector
    d = np.diag(L).copy()
    L[np.diag_indices(P)] = np.exp(d)
    LT = L.T  # lt[k, i] = L[i, k]

    ltimg = np.empty((128, LT_W), dtype=f16)
    for (k0, i0, i1), off in zip(LT_SEGS, LT_OFF):
        ltimg[:, off:off + (i1 - i0)] = LT[k0:k0 + 128, i0:i1]

    m2img = np.zeros((128, 256), dtype=f16)
    m2img[0, :] = mean[512:768].astype(f16)

    z2 = z.reshape(P, S, B)
    in_maps = []
    for c in range(NCORES):
        zs = z2[:, :, c * BC:(c + 1) * BC].reshape(P, NCOL)  # [769, 4096] f32
        xs = x[c * BC:(c + 1) * BC]                           # [256]
        xcol = np.tile(xs, S)                                 # x per column

        zap = np.empty((NCHUNK, 128, 8, CHW), dtype=f16)
        zap[:, :, 0:6, :] = (
            zs[:768].astype(f16).reshape(6, 128, NCHUNK, CHW).transpose(2, 1, 0, 3))
        xz = (xcol[None, :] * zs[0:256]).astype(f16)          # [256, 4096]
        zap[:, :, 6, :] = xz[0:128].reshape(128, NCHUNK, CHW).transpose(1, 0, 2)
        zap[:, :, 7, :] = xz[128:256].reshape(128, NCHUNK, CHW).transpose(1, 0, 2)
        # chunk 0 ships as two column-halves so tiles 0-1 unblock first
        zap0 = np.ascontiguousarray(
            zap[0].reshape(128, 8, 2, 256).transpose(2, 0, 1, 3))  # [2,128,8,256]
        zap = np.ascontiguousarray(zap[1:])

        apm = (xs.reshape(2, 128).T[:, :, None] * mean[None, None, 0:256]
               + mean[None, None, 256:512]).astype(np.float32)  # [128, 2, 256]

        # b1 row handled fully on host: s3[c] = L[768,:] @ z + mean[768]
        s3 = LT[:, 768] @ zs + mean[768]                       # [4096]
        s3img = s3.reshape(NCT, 128).T.astype(np.float32)      # [128, NCT]

        in_maps.append({"za0": zap0, "za": zap, "lt": ltimg,
                        "apm": np.ascontiguousarray(apm),
                        "m2": m2img, "_s3": s3img})
    return in_maps


def _assemble(results, s3imgs):
    out = np.empty((S, B), dtype=np.float32)
    for c in range(NCORES):
        o = results[c]["out"] + s3imgs[c]                    # [128, 32]
        oc = o.reshape(128, S, 2).transpose(1, 2, 0).reshape(S, BC)
        out[:, c * BC:(c + 1) * BC] = oc
    return out


def _run(inputs, trace=False, trace_kwargs=None):
    from concourse.bass_utils import run_bass_kernel_spmd

    if "prog" not in _cache:
        _cache["prog"] = _build_program()
    nc = _cache["prog"]

    in_maps = _prep_inputs(**inputs)
    s3imgs = [im.pop("_s3") for im in in_maps]
    kw = {}
    if trace:
        kw["trace"] = True
        if trace_kwargs:
            kw.update(trace_kwargs)
    res = run_bass_kernel_spmd(nc, in_maps, core_ids=list(range(NCORES)), **kw)
    return _assemble(res.results, s3imgs), res


def kernel(x, mean, cov_vector, z):
    out, _ = _run(dict(x=np.asarray(x), mean=np.asarray(mean),
                       cov_vector=np.asarray(cov_vector), z=np.asarray(z)))
    return out


# revision 20
# speedup vs baseline: 1.0634x; 1.0634x over previous
"""Trainium2 Bass kernel for nn_FCVI_Net_78864189489850.

Computation (reference):
  L = lower-tri scatter of cov_vector (exp on diag)           [769, 769]
  samples = mean + L @ z                                      [769, S, B]
  W0 = samples[0:256], b0 = samples[256:512],
  W1 = samples[512:768], b1 = samples[768]
  out = sum_o relu(x*W0 + b0) * W1 + b1                       [S, B]

Strategy (8 NeuronCores, batch-sharded, no cross-device comms):
  - Everything is fused into ONE PSUM bank per 128-column tile:
      pub[:, 0:256]   = u2 = x*s0 + s1          (via host-prescaled xz rows)
      pub[:, 256:512] = v  = s2 + mean2         (mean2 via a K=1 ones matmul)
  - 9 matmuls per c-tile with block-triangular trimming (2944 cycles):
      z-t0..t2 -> N=512, z-t3 -> N=384, xz-t0 -> N=256, xz-t1 -> N=128,
      ones x mean2 -> N=256, z-t4 -> N=256, z-t5 -> N=128.
  - DVE: u2a = pub0 + apar (apar = x*mean0+mean1, host-built, 2 parities),
         g = max(u2a, 0) * pub1 with accum_out -> stag column  (relu fused!)
  - b1 row (s3 = L[768,:]@z + mean[768]) is computed fully on host and
    added after the device pass; scalar engine and gpsimd are unused.
  - z ships as f16 in DMA-friendly [chunk][part][slot][col] layout
    (8KB contiguous per partition per chunk); LT ships as one packed
    [128, 2688] f16 image of exactly the needed column ranges.
  - ~8 junk warm-up matmuls at t=0 (no DMA deps) spin the PE HAM clock
    up to 2.4 GHz while the first z chunk streams in.
"""
import os
import numpy as np

P = 769
S = 16
B = 2048
NCORES = 8
BC = B // NCORES          # 256 batch per core
NCOL = S * BC             # 4096 columns per core
NCT = NCOL // 128         # 32 c-tiles per core
NCHUNK = 8                # z DMA chunks
CHW = NCOL // NCHUNK      # 512

NWARM = int(os.environ.get("BASS_FCVI_WARM", "13"))

# lt image segments: (k0, i0, i1) in MM issue order; offsets accumulate
LT_SEGS = [
    (0,   256, 768),   # z-t0  -> pub[:, 0:512]
    (128, 256, 768),   # z-t1  -> pub[:, 0:512]
    (256, 256, 768),   # z-t2  -> pub[:, 0:512]
    (384, 384, 768),   # z-t3  -> pub[:, 128:512]
    (512, 512, 768),   # z-t4  -> pub[:, 256:512]
    (640, 640, 768),   # z-t5  -> pub[:, 384:512]
    (0,   0,   256),   # xz-t0 -> pub[:, 0:256]
    (128, 128, 256),   # xz-t1 -> pub[:, 128:256]
]
LT_OFF = []
_o = 0
for _k0, _i0, _i1 in LT_SEGS:
    LT_OFF.append(_o)
    _o += _i1 - _i0
LT_W = _o  # 2688

_cache = {}


def _build_program():
    import concourse.bacc as bacc
    import concourse.tile as tile
    from concourse import mybir

    f16 = mybir.dt.float16
    f32 = mybir.dt.float32

    nc = bacc.Bacc("TRN2", target_bir_lowering=False, debug=False)

    za0_d = nc.dram_tensor("za0", [2, 128, 8, 256], f16, kind="ExternalInput")
    za_d = nc.dram_tensor("za", [NCHUNK - 1, 128, 8, CHW], f16, kind="ExternalInput")
    lt_d = nc.dram_tensor("lt", [128, LT_W], f16, kind="ExternalInput")
    ap_d = nc.dram_tensor("apm", [128, 2, 256], f32, kind="ExternalInput")
    m2_d = nc.dram_tensor("m2", [128, 256], f16, kind="ExternalInput")
    out_d = nc.dram_tensor("out", [128, NCT], f32, kind="ExternalOutput")
    LT_HEAD = 1920  # lt cols for z-t0..t3 (tile 0's first four matmuls)

    with tile.TileContext(nc) as tc:
        with (
            tc.tile_pool(name="zpool", bufs=1) as zpool,
            tc.tile_pool(name="cpool", bufs=1) as cpool,
            tc.tile_pool(name="upool", bufs=3) as upool,
            tc.tile_pool(name="gpool", bufs=2) as gpool,
            tc.tile_pool(name="pub", bufs=7, space="PSUM") as pub_pool,
            tc.tile_pool(name="pwarm", bufs=1, space="PSUM") as pwarm_pool,
        ):
            # --- PE warm-up: junk matmuls with no DMA deps spin HAM to 2.4GHz
            warm = cpool.tile([128, 384], f16, tag="warm")
            nc.vector.memset(warm[:], 0.0)
            # e0: row 0 = ones, rest 0 -> lhsT for the mean2 broadcast matmul
            e0 = cpool.tile([128, 128], f16, tag="e0")
            nc.vector.memset(e0[:], 0.0)
            nc.vector.memset(e0[0:1, :], 1.0)
            pw = pwarm_pool.tile([128, 512], f32, tag="pw")
            for _ in range(NWARM):
                nc.tensor.matmul(pw[:, 0:256], warm[:, 0:128], warm[:, 128:384],
                                 start=True, stop=True)

            # --- DMAs: tile 0's deps first, issue spread over two queues ---
            zc = [None] * NCHUNK
            zc0 = [None, None]
            ltt = cpool.tile([128, LT_W], f16, tag="lt")
            apm = cpool.tile([128, 2, 256], f32, tag="apm")
            m2t = cpool.tile([128, 256], f16, tag="m2")

            for h in range(2):
                zht = zpool.tile([128, 8, 256], f16, tag=f"zc0{h}")
                zc0[h] = zht
            for q in range(1, NCHUNK):
                zqt = zpool.tile([128, 8, CHW], f16, tag=f"zc{q}")
                zc[q] = zqt

            # One queue, strict priority order, big contiguous transfers
            # (every DMA is striped over all 16 DMA engines; multiple queues
            # only steal bandwidth from each other and delay completions).
            nc.sync.dma_start(out=ltt[:], in_=lt_d.ap()[:, :])
            nc.sync.dma_start(out=zc0[0][:], in_=za0_d.ap()[0])
            nc.sync.dma_start(out=zc0[1][:], in_=za0_d.ap()[1])
            nc.sync.dma_start(out=m2t[:], in_=m2_d.ap()[:, :])
            nc.sync.dma_start(out=apm[:], in_=ap_d.ap()[:, :, :])
            for q in range(1, NCHUNK):
                nc.sync.dma_start(out=zc[q][:], in_=za_d.ap()[q - 1, :, :, :])

            stag = cpool.tile([128, NCT], f32, tag="stag")

            def lhs(m, s):
                q, cl = divmod(m * 128, CHW)
                if q == 0:
                    return zc0[m // 2][:, s, (m % 2) * 128:(m % 2) * 128 + 128]
                return zc[q][:, s, cl:cl + 128]

            def seg(t):
                return ltt[:, LT_OFF[t]:LT_OFF[t] + (LT_SEGS[t][2] - LT_SEGS[t][1])]

            MM = nc.tensor.matmul
            for m in range(NCT):
                pub = pub_pool.tile([128, 512], f32, tag="pub")
                MM(pub[:, 0:512],   lhs(m, 0), seg(0), start=True, stop=False)
                MM(pub[:, 0:512],   lhs(m, 1), seg(1), start=False, stop=False)
                MM(pub[:, 0:512],   lhs(m, 2), seg(2), start=False, stop=False)
                MM(pub[:, 128:512], lhs(m, 3), seg(3), start=False, stop=False)
                MM(pub[:, 0:256],   lhs(m, 6), seg(6), start=False, stop=False)
                MM(pub[:, 128:256], lhs(m, 7), seg(7), start=False, stop=False)
                MM(pub[:, 256:512], e0[:, :], m2t[:, :], start=False, stop=False)
                MM(pub[:, 256:512], lhs(m, 4), seg(4), start=False, stop=False)
                MM(pub[:, 384:512], lhs(m, 5), seg(5), start=False, stop=True)

                u2 = upool.tile([128, 256], f32, tag="u2")
                nc.vector.tensor_add(u2[:], pub[:, 0:256], apm[:, m % 2, :])
                g = gpool.tile([128, 256], f32, tag="g")
                nc.vector.scalar_tensor_tensor(
                    out=g[:], in0=u2[:], scalar=0.0, in1=pub[:, 256:512],
                    op0=mybir.AluOpType.max, op1=mybir.AluOpType.mult,
                    accum_out=stag[:, m:m + 1])

                if m in (NCT // 2 - 1, NCT - 1):
                    h_ = 0 if m == NCT // 2 - 1 else 1
                    sl = slice(h_ * (NCT // 2), (h_ + 1) * (NCT // 2))
                    nc.sync.dma_start(out=out_d.ap()[:, sl], in_=stag[:, sl])

    nc.compile()
    return nc


def _prep_inputs(x, mean, cov_vector, z):
    f16 = np.float16

    L = np.zeros((P, P), dtype=np.float32)
    L[np.tril_indices(P)] = cov_vector
    d = np.diag(L).copy()
    L[np.diag_indices(P)] = np.exp(d)
    LT = L.T  # lt[k, i] = L[i, k]

    ltimg = np.empty((128, LT_W), dtype=f16)
    for (k0, i0, i1), off in zip(LT_SEGS, LT_OFF):
        ltimg[:, off:off + (i1 - i0)] = LT[k0:k0 + 128, i0:i1]

    m2img = np.zeros((128, 256), dtype=f16)
    m2img[0, :] = mean[512:768].astype(f16)

    z2 = z.reshape(P, S, B)
    in_maps = []
    for c in range(NCORES):
        zs = z2[:, :, c * BC:(c + 1) * BC].reshape(P, NCOL)  # [769, 4096] f32
        xs = x[c * BC:(c + 1) * BC]                           # [256]
        xcol = np.tile(xs, S)                                 # x per column

        zap = np.empty((NCHUNK, 128, 8, CHW), dtype=f16)
        zap[:, :, 0:6, :] = (
            zs[:768].astype(f16).reshape(6, 128, NCHUNK, CHW).transpose(2, 1, 0, 3))
        xz = (xcol[None, :] * zs[0:256]).astype(f16)          # [256, 4096]
        zap[:, :, 6, :] = xz[0:128].reshape(128, NCHUNK, CHW).transpose(1, 0, 2)
        zap[:, :, 7, :] = xz[128:256].reshape(128, NCHUNK, CHW).transpose(1, 0, 2)
        # chunk 0 ships as two column-halves so tiles 0-1 unblock first
        zap0 = np.ascontiguousarray(
            zap[0].reshape(128, 8, 2, 256).transpose(2, 0, 1, 3))  # [2,128,8,256]
        zap = np.ascontiguousarray(zap[1:])

        apm = (xs.reshape(2, 128).T[:, :, None] * mean[None, None, 0:256]
               + mean[None, None, 256:512]).astype(np.float32)  # [128, 2, 256]

        # b1 row handled fully on host: s3[c] = L[768,:] @ z + mean[768]
        s3 = LT[:, 768] @ zs + mean[768]                       # [4096]
        s3img = s3.reshape(NCT, 128).T.astype(np.float32)      # [128, NCT]

        in_maps.append({"za0": zap0, "za": zap, "lt": ltimg,
                        "apm": np.ascontiguousarray(apm),
                        "m2": m2img, "_s3": s3img})
    return in_maps


def _assemble(results, s3imgs):
    out = np.empty((S, B), dtype=np.float32)
    for c in range(NCORES):
        o = results[c]["out"] + s3imgs[c]                    # [128, 32]
        oc = o.reshape(128, S, 2).transpose(1, 2, 0).reshape(S, BC)
        out[:, c * BC:(c + 1) * BC] = oc
    return out


def _run(inputs, trace=False, trace_kwargs=None):
    from concourse.bass_utils import run_bass_kernel_spmd

    if "prog" not in _cache:
        _cache["prog"] = _build_program()
    nc = _cache["prog"]

    in_maps = _prep_inputs(**inputs)
    s3imgs = [im.pop("_s3") for im in in_maps]
    kw = {}
    if trace:
        kw["trace"] = True
        if trace_kwargs:
            kw.update(trace_kwargs)
    res = run_bass_kernel_spmd(nc, in_maps, core_ids=list(range(NCORES)), **kw)
    return _assemble(res.results, s3imgs), res


def kernel(x, mean, cov_vector, z):
    out, _ = _run(dict(x=np.asarray(x), mean=np.asarray(mean),
                       cov_vector=np.asarray(cov_vector), z=np.asarray(z)))
    return out


# revision 22
# speedup vs baseline: 1.1263x; 1.0591x over previous
"""Trainium2 Bass kernel for nn_FCVI_Net_78864189489850.

Computation (reference):
  L = lower-tri scatter of cov_vector (exp on diag)           [769, 769]
  samples = mean + L @ z                                      [769, S, B]
  W0 = samples[0:256], b0 = samples[256:512],
  W1 = samples[512:768], b1 = samples[768]
  out = sum_o relu(x*W0 + b0) * W1 + b1                       [S, B]

Strategy (8 NeuronCores, batch-sharded, no cross-device comms):
  - Everything is fused into ONE PSUM bank per 128-column tile:
      pub[:, 0:256]   = u2 = x*s0 + s1          (via host-prescaled xz rows)
      pub[:, 256:512] = v  = s2 + mean2         (mean2 via a K=1 ones matmul)
  - 9 matmuls per c-tile with block-triangular trimming (2944 cycles):
      z-t0..t2 -> N=512, z-t3 -> N=384, xz-t0 -> N=256, xz-t1 -> N=128,
      ones x mean2 -> N=256, z-t4 -> N=256, z-t5 -> N=128.
  - DVE: u2a = pub0 + apar (apar = x*mean0+mean1, host-built, 2 parities),
         g = max(u2a, 0) * pub1 with accum_out -> stag column  (relu fused!)
  - b1 row (s3 = L[768,:]@z + mean[768]) is computed fully on host and
    added after the device pass; scalar engine and gpsimd are unused.
  - z ships as f16 in DMA-friendly [chunk][part][slot][col] layout
    (8KB contiguous per partition per chunk); LT ships as one packed
    [128, 2688] f16 image of exactly the needed column ranges.
  - ~8 junk warm-up matmuls at t=0 (no DMA deps) spin the PE HAM clock
    up to 2.4 GHz while the first z chunk streams in.
"""
import os
import numpy as np

P = 769
S = 16
B = 2048
NCORES = 8
BC = B // NCORES          # 256 batch per core
NCOL = S * BC             # 4096 columns per core
NCT = NCOL // 128         # 32 c-tiles per core
NCHUNK = 8                # z DMA chunks
CHW = NCOL // NCHUNK      # 512

NWARM = int(os.environ.get("BASS_FCVI_WARM", "20"))

# lt image segments: (k0, i0, i1) in MM issue order; offsets accumulate
LT_SEGS = [
    (0,   256, 768),   # z-t0  -> pub[:, 0:512]
    (128, 256, 768),   # z-t1  -> pub[:, 0:512]
    (256, 256, 768),   # z-t2  -> pub[:, 0:512]
    (384, 384, 768),   # z-t3  -> pub[:, 128:512]
    (512, 512, 768),   # z-t4  -> pub[:, 256:512]
    (640, 640, 768),   # z-t5  -> pub[:, 384:512]
    (0,   0,   256),   # xz-t0 -> pub[:, 0:256]
    (128, 128, 256),   # xz-t1 -> pub[:, 128:256]
]
LT_OFF = []
_o = 0
for _k0, _i0, _i1 in LT_SEGS:
    LT_OFF.append(_o)
    _o += _i1 - _i0
LT_W = _o  # 2688

_cache = {}


def _build_program():
    import concourse.bacc as bacc
    import concourse.tile as tile
    from concourse import mybir

    f16 = mybir.dt.float16
    f32 = mybir.dt.float32

    nc = bacc.Bacc("TRN2", target_bir_lowering=False, debug=False)

    za0_d = nc.dram_tensor("za0", [2, 128, 8, 256], f16, kind="ExternalInput")
    za_d = nc.dram_tensor("za", [NCHUNK - 1, 128, 8, CHW], f16, kind="ExternalInput")
    lt_d = nc.dram_tensor("lt", [128, LT_W], f16, kind="ExternalInput")
    ap_d = nc.dram_tensor("apm", [128, 2, 256], f32, kind="ExternalInput")
    m2_d = nc.dram_tensor("m2", [128, 256], f16, kind="ExternalInput")
    out_d = nc.dram_tensor("out", [128, NCT], f32, kind="ExternalOutput")
    LT_HEAD = 1920  # lt cols for z-t0..t3 (tile 0's first four matmuls)

    with tile.TileContext(nc) as tc:
        with (
            tc.tile_pool(name="zpool", bufs=1) as zpool,
            tc.tile_pool(name="cpool", bufs=1) as cpool,
            tc.tile_pool(name="upool", bufs=3) as upool,
            tc.tile_pool(name="gpool", bufs=2) as gpool,
            tc.tile_pool(name="pub", bufs=7, space="PSUM") as pub_pool,
            tc.tile_pool(name="pwarm", bufs=1, space="PSUM") as pwarm_pool,
        ):
            # --- PE warm-up: junk matmuls with no DMA deps spin HAM to 2.4GHz
            warm = cpool.tile([128, 384], f16, tag="warm")
            nc.vector.memset(warm[:], 0.0)
            # e0: row 0 = ones, rest 0 -> lhsT for the mean2 broadcast matmul
            e0 = cpool.tile([128, 128], f16, tag="e0")
            nc.vector.memset(e0[:], 0.0)
            nc.vector.memset(e0[0:1, :], 1.0)
            pw = pwarm_pool.tile([128, 512], f32, tag="pw")
            for _ in range(NWARM):
                nc.tensor.matmul(pw[:, 0:256], warm[:, 0:128], warm[:, 128:384],
                                 start=True, stop=True)

            # --- DMAs: tile 0's deps first, issue spread over two queues ---
            zc = [None] * NCHUNK
            zc0 = [None, None]
            ltt = cpool.tile([128, LT_W], f16, tag="lt")
            apm = cpool.tile([128, 2, 256], f32, tag="apm")
            m2t = cpool.tile([128, 256], f16, tag="m2")

            for h in range(2):
                zht = zpool.tile([128, 8, 256], f16, tag=f"zc0{h}")
                zc0[h] = zht
            for q in range(1, NCHUNK):
                zqt = zpool.tile([128, 8, CHW], f16, tag=f"zc{q}")
                zc[q] = zqt

            # One queue, strict priority order, big contiguous transfers
            # (every DMA is striped over all 16 DMA engines; multiple queues
            # only steal bandwidth from each other and delay completions).
            nc.sync.dma_start(out=zc0[0][:], in_=za0_d.ap()[0])
            nc.sync.dma_start(out=ltt[:, 0:LT_HEAD], in_=lt_d.ap()[:, 0:LT_HEAD])
            nc.sync.dma_start(out=ltt[:, LT_HEAD:LT_W],
                              in_=lt_d.ap()[:, LT_HEAD:LT_W])
            nc.sync.dma_start(out=m2t[:], in_=m2_d.ap()[:, :])
            nc.sync.dma_start(out=zc0[1][:], in_=za0_d.ap()[1])
            nc.sync.dma_start(out=apm[:], in_=ap_d.ap()[:, :, :])
            for q in range(1, NCHUNK):
                nc.sync.dma_start(out=zc[q][:], in_=za_d.ap()[q - 1, :, :, :])

            stag = cpool.tile([128, NCT], f32, tag="stag")

            def lhs(m, s):
                q, cl = divmod(m * 128, CHW)
                if q == 0:
                    return zc0[m // 2][:, s, (m % 2) * 128:(m % 2) * 128 + 128]
                return zc[q][:, s, cl:cl + 128]

            def seg(t):
                return ltt[:, LT_OFF[t]:LT_OFF[t] + (LT_SEGS[t][2] - LT_SEGS[t][1])]

            MM = nc.tensor.matmul
            for m in range(NCT):
                pub = pub_pool.tile([128, 512], f32, tag="pub")
                MM(pub[:, 0:512],   lhs(m, 0), seg(0), start=True, stop=False)
                MM(pub[:, 0:512],   lhs(m, 1), seg(1), start=False, stop=False)
                MM(pub[:, 0:512],   lhs(m, 2), seg(2), start=False, stop=False)
                MM(pub[:, 128:512], lhs(m, 3), seg(3), start=False, stop=False)
                MM(pub[:, 0:256],   lhs(m, 6), seg(6), start=False, stop=False)
                MM(pub[:, 128:256], lhs(m, 7), seg(7), start=False, stop=False)
                MM(pub[:, 256:512], e0[:, :], m2t[:, :], start=False, stop=False)
                MM(pub[:, 256:512], lhs(m, 4), seg(4), start=False, stop=False)
                MM(pub[:, 384:512], lhs(m, 5), seg(5), start=False, stop=True)

                u2 = upool.tile([128, 256], f32, tag="u2")
                nc.vector.tensor_add(u2[:], pub[:, 0:256], apm[:, m % 2, :])
                g = gpool.tile([128, 256], f32, tag="g")
                nc.vector.scalar_tensor_tensor(
                    out=g[:], in0=u2[:], scalar=0.0, in1=pub[:, 256:512],
                    op0=mybir.AluOpType.max, op1=mybir.AluOpType.mult,
                    accum_out=stag[:, m:m + 1])

                if m in (NCT // 2 - 1, NCT - 1):
                    h_ = 0 if m == NCT // 2 - 1 else 1
                    sl = slice(h_ * (NCT // 2), (h_ + 1) * (NCT // 2))
                    nc.sync.dma_start(out=out_d.ap()[:, sl], in_=stag[:, sl])

    nc.compile()
    return nc


def _prep_inputs(x, mean, cov_vector, z):
    f16 = np.float16

    L = np.zeros((P, P), dtype=np.float32)
    L[np.tril_indices(P)] = cov_vector
    d = np.diag(L).copy()
    L[np.diag_indices(P)] = np.exp(d)
    LT = L.T  # lt[k, i] = L[i, k]

    ltimg = np.empty((128, LT_W), dtype=f16)
    for (k0, i0, i1), off in zip(LT_SEGS, LT_OFF):
        ltimg[:, off:off + (i1 - i0)] = LT[k0:k0 + 128, i0:i1]

    m2img = np.zeros((128, 256), dtype=f16)
    m2img[0, :] = mean[512:768].astype(f16)

    z2 = z.reshape(P, S, B)
    in_maps = []
    for c in range(NCORES):
        zs = z2[:, :, c * BC:(c + 1) * BC].reshape(P, NCOL)  # [769, 4096] f32
        xs = x[c * BC:(c + 1) * BC]                           # [256]
        xcol = np.tile(xs, S)                                 # x per column

        zap = np.empty((NCHUNK, 128, 8, CHW), dtype=f16)
        zap[:, :, 0:6, :] = (
            zs[:768].astype(f16).reshape(6, 128, NCHUNK, CHW).transpose(2, 1, 0, 3))
        xz = (xcol[None, :] * zs[0:256]).astype(f16)          # [256, 4096]
        zap[:, :, 6, :] = xz[0:128].reshape(128, NCHUNK, CHW).transpose(1, 0, 2)
        zap[:, :, 7, :] = xz[128:256].reshape(128, NCHUNK, CHW).transpose(1, 0, 2)
        # chunk 0 ships as two column-halves so tiles 0-1 unblock first
        zap0 = np.ascontiguousarray(
            zap[0].reshape(128, 8, 2, 256).transpose(2, 0, 1, 3))  # [2,128,8,256]
        zap = np.ascontiguousarray(zap[1:])

        apm = (xs.reshape(2, 128).T[:, :, None] * mean[None, None, 0:256]
               + mean[None, None, 256:512]).astype(np.float32)  # [128, 2, 256]

        # b1 row handled fully on host: s3[c] = L[768,:] @ z + mean[768]
        s3 = LT[:, 768] @ zs + mean[768]                       # [4096]
        s3img = s3.reshape(NCT, 128).T.astype(np.float32)      # [128, NCT]

        in_maps.append({"za0": zap0, "za": zap, "lt": ltimg,
                        "apm": np.ascontiguousarray(apm),
                        "m2": m2img, "_s3": s3img})
    return in_maps


def _assemble(results, s3imgs):
    out = np.empty((S, B), dtype=np.float32)
    for c in range(NCORES):
        o = results[c]["out"] + s3imgs[c]                    # [128, 32]
        oc = o.reshape(128, S, 2).transpose(1, 2, 0).reshape(S, BC)
        out[:, c * BC:(c + 1) * BC] = oc
    return out


def _run(inputs, trace=False, trace_kwargs=None):
    from concourse.bass_utils import run_bass_kernel_spmd

    if "prog" not in _cache:
        _cache["prog"] = _build_program()
    nc = _cache["prog"]

    in_maps = _prep_inputs(**inputs)
    s3imgs = [im.pop("_s3") for im in in_maps]
    kw = {}
    if trace:
        kw["trace"] = True
        if trace_kwargs:
            kw.update(trace_kwargs)
    res = run_bass_kernel_spmd(nc, in_maps, core_ids=list(range(NCORES)), **kw)
    return _assemble(res.results, s3imgs), res


def kernel(x, mean, cov_vector, z):
    out, _ = _run(dict(x=np.asarray(x), mean=np.asarray(mean),
                       cov_vector=np.asarray(cov_vector), z=np.asarray(z)))
    return out


# revision 23
# speedup vs baseline: 1.2177x; 1.0812x over previous
"""Trainium2 Bass kernel for nn_FCVI_Net_78864189489850.

Computation (reference):
  L = lower-tri scatter of cov_vector (exp on diag)           [769, 769]
  samples = mean + L @ z                                      [769, S, B]
  W0 = samples[0:256], b0 = samples[256:512],
  W1 = samples[512:768], b1 = samples[768]
  out = sum_o relu(x*W0 + b0) * W1 + b1                       [S, B]

Strategy (8 NeuronCores, batch-sharded, no cross-device comms):
  - Everything is fused into ONE PSUM bank per 128-column tile:
      pub[:, 0:256]   = u2 = x*s0 + s1          (via host-prescaled xz rows)
      pub[:, 256:512] = v  = s2 + mean2         (mean2 via a K=1 ones matmul)
  - 9 matmuls per c-tile with block-triangular trimming (2944 cycles):
      z-t0..t2 -> N=512, z-t3 -> N=384, xz-t0 -> N=256, xz-t1 -> N=128,
      ones x mean2 -> N=256, z-t4 -> N=256, z-t5 -> N=128.
  - DVE: u2a = pub0 + apar (apar = x*mean0+mean1, host-built, 2 parities),
         g = max(u2a, 0) * pub1 with accum_out -> stag column  (relu fused!)
  - b1 row (s3 = L[768,:]@z + mean[768]) is computed fully on host and
    added after the device pass; scalar engine and gpsimd are unused.
  - z ships as f16 in DMA-friendly [chunk][part][slot][col] layout
    (8KB contiguous per partition per chunk); LT ships as one packed
    [128, 2688] f16 image of exactly the needed column ranges.
  - ~8 junk warm-up matmuls at t=0 (no DMA deps) spin the PE HAM clock
    up to 2.4 GHz while the first z chunk streams in.
"""
import os
import numpy as np

P = 769
S = 16
B = 2048
NCORES = 8
BC = B // NCORES          # 256 batch per core
NCOL = S * BC             # 4096 columns per core
NCT = NCOL // 128         # 32 c-tiles per core
NCHUNK = 8                # z DMA chunks
CHW = NCOL // NCHUNK      # 512

NWARM = int(os.environ.get("BASS_FCVI_WARM", "20"))

# lt image segments: (k0, i0, i1) in MM issue order; offsets accumulate
LT_SEGS = [
    (0,   256, 768),   # z-t0  -> pub[:, 0:512]
    (128, 256, 768),   # z-t1  -> pub[:, 0:512]
    (256, 256, 768),   # z-t2  -> pub[:, 0:512]
    (384, 384, 768),   # z-t3  -> pub[:, 128:512]
    (512, 512, 768),   # z-t4  -> pub[:, 256:512]
    (640, 640, 768),   # z-t5  -> pub[:, 384:512]
    (0,   0,   256),   # xz-t0 -> pub[:, 0:256]
    (128, 128, 256),   # xz-t1 -> pub[:, 128:256]
]
LT_OFF = []
_o = 0
for _k0, _i0, _i1 in LT_SEGS:
    LT_OFF.append(_o)
    _o += _i1 - _i0
LT_W = _o  # 2688

_cache = {}


def _build_program():
    import concourse.bacc as bacc
    import concourse.tile as tile
    from concourse import mybir

    f16 = mybir.dt.float16
    f32 = mybir.dt.float32

    nc = bacc.Bacc("TRN2", target_bir_lowering=False, debug=False)

    za0_d = nc.dram_tensor("za0", [2, 128, 8, 256], f16, kind="ExternalInput")
    za_d = nc.dram_tensor("za", [NCHUNK - 1, 128, 8, CHW], f16, kind="ExternalInput")
    lt_d = nc.dram_tensor("lt", [128, LT_W], f16, kind="ExternalInput")
    ap_d = nc.dram_tensor("apm", [128, 2, 512], f32, kind="ExternalInput")
    out_d = nc.dram_tensor("out", [128, NCT], f32, kind="ExternalOutput")
    LT_HEAD = 1920  # lt cols for z-t0..t3 (tile 0's first four matmuls)

    with tile.TileContext(nc) as tc:
        with (
            tc.tile_pool(name="zpool", bufs=1) as zpool,
            tc.tile_pool(name="cpool", bufs=1) as cpool,
            tc.tile_pool(name="upool", bufs=3) as upool,
            tc.tile_pool(name="gpool", bufs=2) as gpool,
            tc.tile_pool(name="pub", bufs=7, space="PSUM") as pub_pool,
            tc.tile_pool(name="pwarm", bufs=1, space="PSUM") as pwarm_pool,
        ):
            # --- PE warm-up: junk matmuls with no DMA deps spin HAM to 2.4GHz
            warm = cpool.tile([128, 384], f16, tag="warm")
            nc.vector.memset(warm[:], 0.0)
            pw = pwarm_pool.tile([128, 512], f32, tag="pw")
            for _ in range(NWARM):
                nc.tensor.matmul(pw[:, 0:256], warm[:, 0:128], warm[:, 128:384],
                                 start=True, stop=True)

            # --- DMAs: tile 0's deps first, issue spread over two queues ---
            zc = [None] * NCHUNK
            zc0 = [None, None]
            ltt = cpool.tile([128, LT_W], f16, tag="lt")
            apm = cpool.tile([128, 2, 512], f32, tag="apm")

            for h in range(2):
                zht = zpool.tile([128, 8, 256], f16, tag=f"zc0{h}")
                zc0[h] = zht
            for q in range(1, NCHUNK):
                zqt = zpool.tile([128, 8, CHW], f16, tag=f"zc{q}")
                zc[q] = zqt

            # One queue, strict priority order, big contiguous transfers
            # (every DMA is striped over all 16 DMA engines; multiple queues
            # only steal bandwidth from each other and delay completions).
            nc.sync.dma_start(out=zc0[0][:], in_=za0_d.ap()[0])
            nc.sync.dma_start(out=ltt[:, 0:LT_HEAD], in_=lt_d.ap()[:, 0:LT_HEAD])
            nc.sync.dma_start(out=ltt[:, LT_HEAD:LT_W],
                              in_=lt_d.ap()[:, LT_HEAD:LT_W])
            nc.sync.dma_start(out=zc0[1][:], in_=za0_d.ap()[1])
            nc.sync.dma_start(out=zc[1][:], in_=za_d.ap()[0, :, :, :])
            nc.sync.dma_start(out=apm[:], in_=ap_d.ap()[:, :, :])
            for q in range(2, NCHUNK):
                nc.sync.dma_start(out=zc[q][:], in_=za_d.ap()[q - 1, :, :, :])

            stag = cpool.tile([128, NCT], f32, tag="stag")

            def lhs(m, s):
                q, cl = divmod(m * 128, CHW)
                if q == 0:
                    return zc0[m // 2][:, s, (m % 2) * 128:(m % 2) * 128 + 128]
                return zc[q][:, s, cl:cl + 128]

            def seg(t):
                return ltt[:, LT_OFF[t]:LT_OFF[t] + (LT_SEGS[t][2] - LT_SEGS[t][1])]

            MM = nc.tensor.matmul
            for m in range(NCT):
                pub = pub_pool.tile([128, 512], f32, tag="pub")
                MM(pub[:, 0:512],   lhs(m, 0), seg(0), start=True, stop=False)
                MM(pub[:, 0:512],   lhs(m, 1), seg(1), start=False, stop=False)
                MM(pub[:, 0:512],   lhs(m, 2), seg(2), start=False, stop=False)
                MM(pub[:, 128:512], lhs(m, 3), seg(3), start=False, stop=False)
                MM(pub[:, 0:256],   lhs(m, 6), seg(6), start=False, stop=False)
                MM(pub[:, 128:256], lhs(m, 7), seg(7), start=False, stop=False)
                MM(pub[:, 256:512], lhs(m, 4), seg(4), start=False, stop=False)
                MM(pub[:, 384:512], lhs(m, 5), seg(5), start=False, stop=True)

                u2 = upool.tile([128, 512], f32, tag="u2")
                nc.vector.tensor_add(u2[:], pub[:, 0:512], apm[:, m % 2, :])
                g = gpool.tile([128, 256], f32, tag="g")
                nc.vector.scalar_tensor_tensor(
                    out=g[:], in0=u2[:, 0:256], scalar=0.0, in1=u2[:, 256:512],
                    op0=mybir.AluOpType.max, op1=mybir.AluOpType.mult,
                    accum_out=stag[:, m:m + 1])

                if m in (NCT // 2 - 1, NCT - 1):
                    h_ = 0 if m == NCT // 2 - 1 else 1
                    sl = slice(h_ * (NCT // 2), (h_ + 1) * (NCT // 2))
                    nc.sync.dma_start(out=out_d.ap()[:, sl], in_=stag[:, sl])

    nc.compile()
    return nc


def _prep_inputs(x, mean, cov_vector, z):
    f16 = np.float16

    L = np.zeros((P, P), dtype=np.float32)
    L[np.tril_indices(P)] = cov_vector
    d = np.diag(L).copy()
    L[np.diag_indices(P)] = np.exp(d)
    LT = L.T  # lt[k, i] = L[i, k]

    ltimg = np.empty((128, LT_W), dtype=f16)
    for (k0, i0, i1), off in zip(LT_SEGS, LT_OFF):
        ltimg[:, off:off + (i1 - i0)] = LT[k0:k0 + 128, i0:i1]


    z2 = z.reshape(P, S, B)
    in_maps = []
    for c in range(NCORES):
        zs = z2[:, :, c * BC:(c + 1) * BC].reshape(P, NCOL)  # [769, 4096] f32
        xs = x[c * BC:(c + 1) * BC]                           # [256]
        xcol = np.tile(xs, S)                                 # x per column

        zap = np.empty((NCHUNK, 128, 8, CHW), dtype=f16)
        zap[:, :, 0:6, :] = (
            zs[:768].astype(f16).reshape(6, 128, NCHUNK, CHW).transpose(2, 1, 0, 3))
        xz = (xcol[None, :] * zs[0:256]).astype(f16)          # [256, 4096]
        zap[:, :, 6, :] = xz[0:128].reshape(128, NCHUNK, CHW).transpose(1, 0, 2)
        zap[:, :, 7, :] = xz[128:256].reshape(128, NCHUNK, CHW).transpose(1, 0, 2)
        # chunk 0 ships as two column-halves so tiles 0-1 unblock first
        zap0 = np.ascontiguousarray(
            zap[0].reshape(128, 8, 2, 256).transpose(2, 0, 1, 3))  # [2,128,8,256]
        zap = np.ascontiguousarray(zap[1:])

        apm = np.empty((128, 2, 512), dtype=np.float32)
        apm[:, :, 0:256] = (xs.reshape(2, 128).T[:, :, None] * mean[None, None, 0:256]
                            + mean[None, None, 256:512])
        apm[:, :, 256:512] = mean[None, None, 512:768]

        # b1 row handled fully on host: s3[c] = L[768,:] @ z + mean[768]
        s3 = LT[:, 768] @ zs + mean[768]                       # [4096]
        s3img = s3.reshape(NCT, 128).T.astype(np.float32)      # [128, NCT]

        in_maps.append({"za0": zap0, "za": zap, "lt": ltimg,
                        "apm": np.ascontiguousarray(apm), "_s3": s3img})
    return in_maps


def _assemble(results, s3imgs):
    out = np.empty((S, B), dtype=np.float32)
    for c in range(NCORES):
        o = results[c]["out"] + s3imgs[c]                    # [128, 32]
        oc = o.reshape(128, S, 2).transpose(1, 2, 0).reshape(S, BC)
        out[:, c * BC:(c + 1) * BC] = oc
    return out


def _run(inputs, trace=False, trace_kwargs=None):
    from concourse.bass_utils import run_bass_kernel_spmd

    if "prog" not in _cache:
        _cache["prog"] = _build_program()
    nc = _cache["prog"]

    in_maps = _prep_inputs(**inputs)
    s3imgs = [im.pop("_s3") for im in in_maps]
    kw = {}
    if trace:
        kw["trace"] = True
        if trace_kwargs:
            kw.update(trace_kwargs)
    res = run_bass_kernel_spmd(nc, in_maps, core_ids=list(range(NCORES)), **kw)
    return _assemble(res.results, s3imgs), res


def kernel(x, mean, cov_vector, z):
    out, _ = _run(dict(x=np.asarray(x), mean=np.asarray(mean),
                       cov_vector=np.asarray(cov_vector), z=np.asarray(z)))
    return out
